# revision 50
# baseline (speedup 1.0000x reference)
"""Trainium2 Bass kernel for cumulative-state (linear) attention over M modalities.

Math (reference): out[i, e] = sum_m sum_{j : t2_m[j] <= t1[i]} (Q[i] . K_m[j]) * X_m[j, e],
for e in {0, 1}, where Q = mlp_q(X[0]), K_m = mlp_km(X[m]), t1 = X[0,:,-1], t2_m = X[m,:,-1].

Sharding: 8 cores = (m, h): modality m in 0..3, key-half h in 0..1. Each core owns
keys j in [h*4096, (h+1)*4096) of modality m and computes partial contributions for
ALL queries; the host scatter-sums the 8 partial outputs (the "all-reduce").

Device-graph structure per core (one static SPMD graph, per-core data):
  - MLPs use BLOCK-DIAGONAL (128x128) stationary weights [[W,0],[0,W]] so one
    full-contract matmul processes both packed 64-row halves per 512-col block.
  - Chunk states S_k = sum_{j in chunk k} K_j (x) V2_j come from kt (x) v2f
    elementwise products (GpSimd, block-pipelined behind the K-MLP) + grouped
    free-dim reduces (DVE). The K-bias term is corrected by a host scorr
    (subtracted); cross-half duplication via two identity matmuls (PSUM bounce).
  - Prefix states via ONE hardware scan op per column (tensor_tensor_scan).
  - Band phase per batch (~3 consecutive same-side 128-key chunks):
    B matmuls into one PSUM tile, ONE mask multiply (DVE), batched po-state +
    po-corr matmuls with stacked stationaries, one staging copy into a single
    stacked SBUF tile, 4 coarse DMAs out (dma_start issue time is ~650ns of
    engine queue each, so descriptor count is minimized everywhere).
  - Tails flipped: qt_block^T @ srun_total -> (128, 2) PSUM cols, 1 copy, 1 DMA.
  - DMA issue: inputs on Sync (weights/xkt first), v2f on GpSimd (software
    DGE), outputs on Sync; Scalar(ACT) and DVE stay compute-only.
"""

import os
from contextlib import ExitStack

import ml_dtypes
import numpy as np

BF16 = ml_dtypes.bfloat16

M, T, D = 4, 8192, 64
NLIN = 3
C = 128          # key chunk size
NK = T // 2      # keys per core (4096)
NKC = NK // C    # local key chunks per core (32)
NSIDE = NKC // 2  # 16 chunks per side
KW = NSIDE * C   # 2048 key cols per side
NCORES = 8
FMAX = 512       # max matmul free dim / PSUM bank cols (f32)
TB = 128         # tail block (stationary free dim)
POMAX = 512      # max cols per batched po matmul
NKB = KW // FMAX  # K-MLP blocks (4)

# combined const layouts
# bf16 const: wk (384) | v2st (64) | wq (384) | wq3T (128) | bq3 (8)
WKOFF, V2OFF, WQOFF, WQ3OFF, BQ3OFF, CBF_W = 0, 384, 448, 832, 960, 968
# f32 const: bk (128,3) | bq (128,3) | ident (128,64) | scorr (128,64)
BKOFF, BQOFF, IDOFF, SCOFF, CF32_W = 0, 3, 6, 70, 134


def _round_up(x, k):
    return ((x + k - 1) // k) * k


def make_plan(X):
    """Host-side: band structure + packed column layout, shared across cores."""
    X = np.asarray(X, np.float32)
    t1 = X[0, :, -1]
    los, his, tbs, idxs = [], [], [], []
    for c in range(NCORES):
        m, h = c // 2, c % 2
        t2 = X[m, :, -1]
        idx = np.searchsorted(t2, t1, side="right") - 1
        idxs.append(idx)
        hs = h * NK
        lo = np.searchsorted(idx, hs + np.arange(NKC) * C, side="left")
        hi = np.searchsorted(idx, hs + (np.arange(NKC) + 1) * C, side="left")
        los.append(lo)
        his.append(hi)
        tbs.append(int(np.searchsorted(idx, hs + NK, side="left")))

    NB = [0] * NKC
    for k in range(NKC):
        w = max(his[c][k] - los[c][k] for c in range(NCORES))
        NB[k] = _round_up(int(w), 8)
        assert NB[k] <= POMAX
    NBAND = int(sum(NB))
    NT = _round_up(max(T - tb for tb in tbs), 8)

    # contiguous side split: chunks 0..15 -> side 0, 16..31 -> side 1
    sideof = [0 if k < NSIDE else 1 for k in range(NKC)]
    kpos = [k % NSIDE for k in range(NKC)]
    qoff = [0] * NKC
    acc = [0, 0]
    for k in range(NKC):
        s = sideof[k]
        qoff[k] = acc[s]
        acc[s] += NB[k]
    lb, rb = acc[0], acc[1]
    # tail split across sides to balance total width
    tL = int(np.clip(_round_up((NBAND + NT) // 2 - lb, 8), 0, NT))
    tR = NT - tL
    NW2 = max(lb + tL, rb + tR)
    toff = [lb, rb]
    tlen = [tL, tR]

    # po batches: consecutive chunks of one side, sum NB <= POMAX, skip NB==0
    batches = []  # (side, k0, ks[], cols)
    for s in range(2):
        ks = [k for k in range(NKC) if sideof[k] == s and NB[k] > 0]
        cur = []
        w = 0
        for k in ks:
            if cur and (w + NB[k] > POMAX or k != cur[-1] + 1):
                batches.append((s, cur[0], list(cur), w))
                cur, w = [], 0
            cur.append(k)
            w += NB[k]
        if cur:
            batches.append((s, cur[0], list(cur), w))

    # tail blocks: (side, col_off_in_qt, width, out2_col_index)
    tblocks = []
    tcol = 0
    for s in range(2):
        o = 0
        while o < tlen[s]:
            wdt = min(TB, tlen[s] - o)
            tblocks.append((s, toff[s] + o, wdt, tcol))
            tcol += 1
            o += wdt
    NTBLK = max(tcol, 1)

    return dict(NB=NB, NBAND=NBAND, NT=NT, NW2=NW2, sideof=sideof, kpos=kpos,
                qoff=qoff, toff=toff, tlen=tlen, los=los, his=his, tbs=tbs,
                idxs=idxs, batches=batches, tblocks=tblocks, NTBLK=NTBLK)


def make_inputs(X, wq_w, wq_b, wk_w, wk_b, plan):
    X = np.asarray(X, np.float32)
    wq_w = np.asarray(wq_w, np.float32)
    wq_b = np.asarray(wq_b, np.float32)
    wk_w = np.asarray(wk_w, np.float32)
    wk_b = np.asarray(wk_b, np.float32)
    NB, NW2 = plan["NB"], plan["NW2"]
    sideof, kpos = plan["sideof"], plan["kpos"]
    qoff, toff, tlen = plan["qoff"], plan["toff"], plan["tlen"]

    def blockdiag(w):  # (NLIN, D, D) -> (128, 128*NLIN)
        cols = []
        for l in range(NLIN):
            b = np.zeros((2 * D, 2 * D), np.float32)
            b[:D, :D] = w[l]
            b[D:, D:] = w[l]
            cols.append(b)
        return np.concatenate(cols, axis=1).astype(BF16)

    wq = blockdiag(wq_w)                                              # (128, 384)
    bq1 = np.stack([wq_b[l] for l in range(NLIN)], axis=1)
    bq = np.concatenate([bq1, bq1], axis=0).astype(np.float32)        # (128, 3)

    ident = np.zeros((2 * D, D), np.float32)                          # [I64; I64]
    ident[:D] = np.eye(D, dtype=np.float32)
    ident[D:] = np.eye(D, dtype=np.float32)

    in_maps = []
    for c in range(NCORES):
        m, h = c // 2, c % 2
        hs = h * NK
        lo, hi, tb = plan["los"][c], plan["his"][c], plan["tbs"][c]
        idx = plan["idxs"][c]

        qb = np.zeros((2 * D, NW2), BF16)
        msk = np.zeros((C, 2 * NW2), BF16)
        for k in range(NKC):
            n = hi[k] - lo[k]
            s, o = sideof[k], qoff[k]
            if n > 0:
                qb[64 * s:64 * s + 64, o:o + n] = X[0, lo[k]:hi[k], :].T.astype(BF16)
                jg = hs + k * C + np.arange(C)[:, None]
                msk[:, s * NW2 + o:s * NW2 + o + n] = \
                    (jg <= idx[None, lo[k]:hi[k]]).astype(BF16)
        # tail: first tlen[0] tail queries on side 0, rest on side 1
        ntail = T - tb
        n0 = min(ntail, tlen[0])
        if n0 > 0:
            qb[0:64, toff[0]:toff[0] + n0] = X[0, tb:tb + n0, :].T.astype(BF16)
        n1 = ntail - n0
        if n1 > 0:
            qb[64:128, toff[1]:toff[1] + n1] = X[0, tb + n0:, :].T.astype(BF16)

        xk = X[m, hs:hs + NK, :]
        xkt = np.zeros((2 * D, KW), BF16)
        v2st = np.zeros((C, 2 * NKC), BF16)
        v2f = np.zeros((2 * D, 2 * KW), BF16)
        for k in range(NKC):
            s, p = sideof[k], kpos[k]
            xkt[64 * s:64 * s + 64, p * C:(p + 1) * C] = \
                xk[k * C:(k + 1) * C, :].T.astype(BF16)
            v2st[:, 2 * k:2 * k + 2] = xk[k * C:(k + 1) * C, 0:2].astype(BF16)
            for e in range(2):
                v2f[64 * s:64 * s + 64, e * KW + p * C:e * KW + (p + 1) * C] = \
                    xk[k * C:(k + 1) * C, e].astype(BF16)[None, :]

        wk = blockdiag(wk_w[m])                                       # (128, 384)
        bk1 = np.stack([wk_b[m, l] for l in range(NLIN)], axis=1)
        bk = np.concatenate([bk1, bk1], axis=0).astype(np.float32)    # (128, 3)

        # S built from kt (WITH final bias) needs b3 (x) sum_j v2 SUBTRACTED.
        b3 = wk_b[m, NLIN - 1]                                        # (64,)
        scorr1 = np.zeros((D, 2 * NKC), np.float32)
        for k in range(NKC):
            vs = np.asarray(v2st[:, 2 * k:2 * k + 2], np.float32).sum(axis=0)
            scorr1[:, 2 * k:2 * k + 2] = b3[:, None] * vs[None, :]
        scorr = np.concatenate([scorr1, scorr1], axis=0)              # (128, 64)

        wq3t1 = wq_w[NLIN - 1].T  # (D, D): lhsT for Wq3 @ srun
        wq3t = np.zeros((2 * D, 2 * D), np.float32)
        wq3t[:D, :D] = wq3t1
        wq3t[D:, D:] = wq3t1

        cbf = np.zeros((2 * D, CBF_W), BF16)
        cbf[:, WKOFF:WKOFF + 384] = wk
        cbf[:, V2OFF:V2OFF + 64] = v2st
        cbf[:, WQOFF:WQOFF + 384] = wq
        cbf[:, WQ3OFF:WQ3OFF + 128] = wq3t.astype(BF16)
        cbf[:, BQ3OFF] = bq[:, NLIN - 1].astype(BF16)
        cbf[:, BQ3OFF + 1] = bq[:, NLIN - 1].astype(BF16)
        cf32 = np.zeros((2 * D, CF32_W), np.float32)
        cf32[:, BKOFF:BKOFF + 3] = bk
        cf32[:, BQOFF:BQOFF + 3] = bq
        cf32[:, IDOFF:IDOFF + 64] = ident
        cf32[:, SCOFF:SCOFF + 64] = scorr

        in_maps.append(dict(qb=qb, msk=msk, xkt=xkt, v2f=v2f,
                            cbf=cbf, cf32=cf32))
    return in_maps


def scatter_outputs(plan, outs, outs2, bq3):
    """Host-side 'all-reduce': scatter per-core stacked band partials
    (16, NBATCH*POMAX) and flipped tail partials (TB, 2*NTBLK+4) to (T, 2).
    bq3 (64,) recovers the tail constant bq3.srun_tot from the exported
    srun_tot columns."""
    NB, NW2 = plan["NB"], plan["NW2"]
    qoff, tlen = plan["qoff"], plan["tlen"]
    y = np.zeros((T, 2), np.float32)
    for c in range(NCORES):
        o = np.asarray(outs[c], np.float32)
        o2 = np.asarray(outs2[c], np.float32)
        lo, hi, tb = plan["los"][c], plan["his"][c], plan["tbs"][c]
        for bi, (s, k0, ks, wsum) in enumerate(plan["batches"]):
            o0 = qoff[k0]
            for gi, k in enumerate(ks):
                n = hi[k] - lo[k]
                if n > 0:
                    col = bi * POMAX + (qoff[k] - o0)
                    y[lo[k]:hi[k], :] += o[2 * gi:2 * gi + 2, col:col + n].T
        ntail = T - tb
        NTBLK = plan["NTBLK"]
        stot = o2[:, 2 * NTBLK:2 * NTBLK + 2]          # (128, 2) srun_tot
        cconst = [bq3 @ stot[0:64], bq3 @ stot[64:128]]
        for (s, coff, w, tc) in plan["tblocks"]:
            slot0 = (coff - plan["toff"][s]) + (0 if s == 0 else tlen[0])
            nn = min(w, max(0, ntail - slot0))
            if nn > 0:
                y[tb + slot0:tb + slot0 + nn, :] += (
                    o2[:nn, 2 * tc:2 * tc + 2] + cconst[s][None, :])
    return y


# ---------------------------------------------------------------- numpy emulation
def emulate_core(im, plan, strict=True):
    """Numpy mirror of the device graph for one core. strict=True models the
    bf16 rounding points of the device graph."""
    NB, NW2 = plan["NB"], plan["NW2"]
    sideof, kpos, qoff = plan["sideof"], plan["kpos"], plan["qoff"]
    toff, tlen = plan["toff"], plan["tlen"]

    def f(x):
        return np.asarray(x, np.float32)

    def rnd(x):  # bf16 round-trip
        return x.astype(BF16).astype(np.float32) if strict else x

    cbf, cf32 = f(im["cbf"]), f(im["cf32"])
    wk = cbf[:, WKOFF:WKOFF + 384]
    v2st = cbf[:, V2OFF:V2OFF + 64]
    wq = cbf[:, WQOFF:WQOFF + 384]
    bk = cf32[:, BKOFF:BKOFF + 3]
    bq = cf32[:, BQOFF:BQOFF + 3]
    scorr = cf32[:, SCOFF:SCOFF + 64]
    qb, xkt, v2f, msk = f(im["qb"]), f(im["xkt"]), f(im["v2f"]), f(im["msk"])

    def mlp_bd(xp, w, b):
        a = xp
        for l in range(NLIN):
            z = w[:, 128 * l:128 * (l + 1)].T @ a + b[:, l][:, None]
            a2 = a
            a = rnd(np.maximum(z, 0.0) if l < NLIN - 1 else z)
        return a, a2

    kt, _ = mlp_bd(xkt, wk, bk)       # (128, KW) bf16
    qt, a2q = mlp_bd(qb, wq, bq)      # (128, NW2) bf16; a2q = layer-2 out

    # S path: product (bf16 round) + grouped reduce, f32 accumulate
    sc2h = np.zeros((2 * D, 2 * NSIDE), np.float32)   # col 2p+e
    for e in range(2):
        prod = rnd(kt * v2f[:, e * KW:(e + 1) * KW])  # (128, KW)
        red = prod.reshape(2 * D, NSIDE, C).sum(axis=2)  # f32 (128, 16)
        sc2h[:, e::2] = red
    # assemble global-order scFull (128, 64) on both halves, minus scorr
    scF = np.zeros((2 * D, 2 * NKC), np.float32)
    scF[0:64, 0:2 * NSIDE] = sc2h[0:64]
    scF[64:128, 2 * NSIDE:] = sc2h[64:128]
    scF[64:128, 0:2 * NSIDE] = sc2h[0:64]     # identity-matmul swap
    scF[0:64, 2 * NSIDE:] = sc2h[64:128]
    scF = scF - scorr
    # exclusive prefix scan -> srun (128, 2*(NKC+1))
    srun = np.zeros((2 * D, 2 * (NKC + 1)), np.float32)
    srun[:, 2:] = np.cumsum(scF.reshape(2 * D, NKC, 2), axis=1).reshape(2 * D, -1)
    srunb = rnd(srun)

    out = np.zeros((2 * NKC // 2, len(plan["batches"]) * POMAX), np.float32)
    for bi, (s, k0, ks, wsum) in enumerate(plan["batches"]):
        o0 = qoff[k0]
        for gi, k in enumerate(ks):
            nq = NB[k]
            o = qoff[k]
            p = kpos[k]
            qblk = qt[64 * s:64 * s + 64, o:o + nq]
            mask = msk[:, s * NW2 + o:s * NW2 + o + nq]
            B = kt[64 * s:64 * s + 64, p * C:(p + 1) * C].T @ qblk
            bm = rnd(B * mask)
            col = bi * POMAX + (o - o0)
            out[2 * gi:2 * gi + 2, col:col + nq] = (
                srunb[64 * s:64 * s + 64, 2 * k:2 * k + 2].T @ qblk
                + v2st[:, 2 * k:2 * k + 2].T @ bm)
    # tails via layer-2 activations: q.srun = a2.(Wq3 srun) + bq3.srun
    wq3t = cbf[:, WQ3OFF:WQ3OFF + 128]
    NTBLK = plan["NTBLK"]
    wsb = rnd(wq3t.T @ srunb[:, 2 * NKC:2 * NKC + 2])        # (128, 2)
    out2 = np.zeros((TB, 2 * NTBLK + 4), np.float32)
    for (s, coff, w, tc) in plan["tblocks"]:
        a2blk = a2q[64 * s:64 * s + 64, coff:coff + w]
        out2[:w, 2 * tc:2 * tc + 2] = a2blk.T @ wsb[64 * s:64 * s + 64, :]
    out2[:, 2 * NTBLK:2 * NTBLK + 2] = srun[:, 2 * NKC:2 * NKC + 2]
    return out, out2


# ---------------------------------------------------------------- device graph
def build_graph(plan):
    import concourse.bacc as bacc
    import concourse.tile as tile
    from concourse import mybir

    NB, NW2, NTBLK = plan["NB"], plan["NW2"], plan["NTBLK"]
    sideof, kpos, qoff = plan["sideof"], plan["kpos"], plan["qoff"]
    toff, tlen = plan["toff"], plan["tlen"]
    NBATCH = len(plan["batches"])
    f32 = mybir.dt.float32
    bf16 = mybir.dt.bfloat16
    AF = mybir.ActivationFunctionType
    OP = mybir.AluOpType

    nc = bacc.Bacc("TRN2")
    d_qb = nc.dram_tensor("qb", [2 * D, NW2], bf16, kind="ExternalInput")
    d_msk = nc.dram_tensor("msk", [C, 2 * NW2], bf16, kind="ExternalInput")
    d_xkt = nc.dram_tensor("xkt", [2 * D, KW], bf16, kind="ExternalInput")
    d_v2f = nc.dram_tensor("v2f", [2 * D, 2 * KW], bf16, kind="ExternalInput")
    d_cbf = nc.dram_tensor("cbf", [2 * D, CBF_W], bf16, kind="ExternalInput")
    d_cf32 = nc.dram_tensor("cf32", [2 * D, CF32_W], f32, kind="ExternalInput")
    d_out = nc.dram_tensor("out", [2 * NKC // 2, NBATCH * POMAX], f32,
                           kind="ExternalOutput")
    d_out2 = nc.dram_tensor("out2", [TB, 2 * NTBLK + 4], f32,
                            kind="ExternalOutput")

    with ExitStack() as ctx:
        tc = ctx.enter_context(tile.TileContext(nc))
        const = ctx.enter_context(tc.tile_pool(name="const", bufs=1))
        big = ctx.enter_context(tc.tile_pool(name="big", bufs=1))
        work = ctx.enter_context(tc.tile_pool(name="work", bufs=1))
        pmlp = ctx.enter_context(tc.tile_pool(name="pmlp", bufs=3, space="PSUM"))
        pb = ctx.enter_context(tc.tile_pool(name="pb", bufs=2, space="PSUM"))
        ppo = ctx.enter_context(tc.tile_pool(name="ppo", bufs=2, space="PSUM"))
        paux = ctx.enter_context(tc.tile_pool(name="paux", bufs=1, space="PSUM"))

        cbf_t = const.tile([2 * D, CBF_W], bf16, tag="cbf")
        cf32_t = const.tile([2 * D, CF32_W], f32, tag="cf32")
        wk_t = cbf_t[:, WKOFF:WKOFF + 384]
        v2st_t = cbf_t[:, V2OFF:V2OFF + 64]
        wq_t = cbf_t[:, WQOFF:WQOFF + 384]
        wq3t_t = cbf_t[:, WQ3OFF:WQ3OFF + 128]
        bk_t = cf32_t[:, BKOFF:BKOFF + 3]
        bq_t = cf32_t[:, BQOFF:BQOFF + 3]
        ident_t = cf32_t[:, IDOFF:IDOFF + 64]
        scorr_t = cf32_t[:, SCOFF:SCOFF + 64]

        xkt_t = big.tile([2 * D, KW], bf16, tag="xkt")
        v2f_t = big.tile([2 * D, 2 * KW], bf16, tag="v2f")
        qb_t = big.tile([2 * D, NW2], bf16, tag="qb")
        msk_t = big.tile([C, 2 * NW2], bf16, tag="msk")

        # ---- input DMA issue: compute-critical first, all on Sync
        # (~650ns issue cost each => few, coarse DMAs; keep ACT/DVE/GP clean)
        nc.sync.dma_start(cbf_t[:, 0:WQOFF], d_cbf[:, 0:WQOFF])
        nc.sync.dma_start(cf32_t[:], d_cf32[:])
        for i in range(NKB):
            nc.sync.dma_start(xkt_t[:, i * FMAX:(i + 1) * FMAX],
                              d_xkt[:, i * FMAX:(i + 1) * FMAX])
        NQB = 4
        qsp = _round_up((NW2 + NQB - 1) // NQB, 8)

        def qb_dma(i):
            a, b = i * qsp, min((i + 1) * qsp, NW2)
            if a < b:
                nc.sync.dma_start(qb_t[:, a:b], d_qb[:, a:b])

        qb_dma(0)
        qb_dma(1)
        nc.sync.dma_start(v2f_t[:, 0:KW // 2], d_v2f[:, 0:KW // 2])
        nc.sync.dma_start(v2f_t[:, KW // 2:KW], d_v2f[:, KW // 2:KW])
        nc.sync.dma_start(cbf_t[:, WQOFF:], d_cbf[:, WQOFF:])
        qb_dma(2)
        qb_dma(3)
        nc.sync.dma_start(v2f_t[:, KW:KW + KW // 2], d_v2f[:, KW:KW + KW // 2])
        nc.sync.dma_start(v2f_t[:, KW + KW // 2:], d_v2f[:, KW + KW // 2:])
        # mask: only the band column ranges are ever read
        lb, rb = toff[0], toff[1]
        for (a, b) in ((0, lb), (NW2, NW2 + rb // 2), (NW2 + rb // 2, NW2 + rb)):
            if a < b:
                nc.sync.dma_start(msk_t[:, a:b], d_msk[:, a:b])

        kt_t = big.tile([2 * D, KW], bf16, tag="kt")
        qt_t = big.tile([2 * D, NW2], bf16, tag="qt")
        a1_t = work.tile([2 * D, NW2], bf16, tag="a1")
        a2_t = work.tile([2 * D, NW2], bf16, tag="a2")
        ktv2_t = big.tile([2 * D, 2 * KW], bf16, tag="ktv2")
        sc2h_t = big.tile([2 * D, 2 * NSIDE], f32, tag="sc2h")
        scF_t = big.tile([2 * D, 2 * NKC], f32, tag="scF")
        srun_t = big.tile([2 * D, 2 * (NKC + 1)], f32, tag="srun")
        srunb_t = big.tile([2 * D, 2 * (NKC + 1)], bf16, tag="srunb")
        bm_t = big.tile([C, 2 * NW2], bf16, tag="bm")
        stg_t = big.tile([2 * NKC // 2, NBATCH * POMAX], f32, tag="stg")

        # ACT table warm + PE warm-up burst, pinned to the front of the
        # engine queues so they run during the DMA window
        with tc.high_priority():
            wup_t = work.tile([C, FMAX], bf16, tag="wup", name="wup")
            nc.vector.memset(wup_t[:], 0.0)
            nc.scalar.activation(wup_t[:, 0:8], wup_t[:, 0:8], AF.Relu)
            for _ in range(5):
                pwu = pb.tile([C, POMAX], f32, tag="pb", name="pwu")
                nc.tensor.matmul(pwu[:, 0:POMAX], wup_t[:, 0:C],
                                 wup_t[:, 0:POMAX], start=True, stop=True)

        # weighted 2-engine epilogue rotation (DVE carries masks/reduces too)
        epil_i = [0]

        def epilogue(dst, src, b_ap, relu):
            i = epil_i[0]
            epil_i[0] += 1
            if (2 * i) % 5 < 2:
                if relu:
                    nc.vector.tensor_scalar(dst, src, b_ap, 0.0, OP.add, OP.max)
                else:
                    nc.vector.tensor_scalar_add(dst, src, b_ap)
            else:
                nc.scalar.activation(dst, src, AF.Relu if relu else AF.Identity,
                                     bias=b_ap)

        def mlp3(src_t, w_t, b_t, n_cols, out_t, block_done=None, l3_cols=None):
            """Block-diagonal 3-layer MLP, layer-major. block_done(bi) fires
            after the LAST layer's epilogue of block bi; l3_cols limits the
            final linear to the leading columns (tails consume layer-2)."""
            stage = [src_t, a1_t, a2_t, out_t]
            for l in range(NLIN):
                nc_l = n_cols if (l < NLIN - 1 or l3_cols is None) else l3_cols
                nblk = (nc_l + FMAX - 1) // FMAX
                for bi in range(nblk):
                    a = bi * FMAX
                    b = min(a + FMAX, nc_l)
                    n = b - a
                    pz = pmlp.tile([C, FMAX], f32, tag="pmlp", name="pz")
                    nc.tensor.matmul(pz[:, :n], w_t[:, 2 * D * l:2 * D * (l + 1)],
                                     stage[l][:, a:b], start=True, stop=True)
                    epilogue(stage[l + 1][:, a:b], pz[:, :n], b_t[:, l:l + 1],
                             l < NLIN - 1)
                    if l == NLIN - 1 and block_done is not None:
                        block_done(bi, a, b)

        # K mlp with the S-product/reduce pipeline chasing its last layer
        # (products split: e=0 on DVE, e=1 on GpSimd; reduces are DVE-only)
        def k_block_done(bi, a, b):
            nc.vector.tensor_mul(ktv2_t[:, a:b], kt_t[:, a:b], v2f_t[:, a:b])
            nc.gpsimd.tensor_mul(ktv2_t[:, KW + a:KW + b], kt_t[:, a:b],
                                 v2f_t[:, KW + a:KW + b])
            g0, g1 = a // C, b // C
            for e in range(2):
                src = ktv2_t[:, e * KW + a:e * KW + b].rearrange(
                    "p (g c) -> p g c", g=g1 - g0)
                nc.vector.reduce_sum(sc2h_t[:, 2 * g0 + e:2 * g1:2], src,
                                     axis=mybir.AxisListType.X)

        mlp3(xkt_t, wk_t, bk_t, KW, kt_t, block_done=k_block_done)

        AUXW = max(2 * NTBLK + 4, 2 * NSIDE)

        # cross-half duplication via identity matmuls (f32, exact)
        def emit_swap():
            psw = paux.tile([2 * D, AUXW], f32, tag="paux", name="psw")
            psw = psw[:, 0:2 * NSIDE]
            nc.tensor.matmul(psw[64:128, :], ident_t[0:64, :], sc2h_t[0:64, :],
                             start=True, stop=True, tile_position=(0, 64))
            nc.tensor.matmul(psw[0:64, :], ident_t[64:128, :], sc2h_t[64:128, :],
                             start=True, stop=True, tile_position=(64, 0))
            # assemble scF (global chunk order, both halves) minus scorr
            nc.gpsimd.tensor_sub(scF_t[0:64, 0:2 * NSIDE], sc2h_t[0:64, :],
                                 scorr_t[0:64, 0:2 * NSIDE])
            nc.gpsimd.tensor_sub(scF_t[64:128, 2 * NSIDE:], sc2h_t[64:128, :],
                                 scorr_t[64:128, 2 * NSIDE:])
            nc.vector.tensor_sub(scF_t[64:128, 0:2 * NSIDE], psw[64:128, :],
                                 scorr_t[64:128, 0:2 * NSIDE])
            nc.vector.tensor_sub(scF_t[0:64, 2 * NSIDE:], psw[0:64, :],
                                 scorr_t[0:64, 2 * NSIDE:])
            # exclusive prefix scan, one HW scan per e-column
            nc.vector.memset(srun_t[:, 0:2], 0.0)
            for e in range(2):
                nc.vector.tensor_tensor_scan(
                    srun_t[:, 2 + e::2], scF_t[:, e::2], scF_t[:, e::2],
                    0.0, OP.add, OP.bypass)
            nc.scalar.copy(srunb_t[:], srun_t[:])

        # Q mlp (final linear only over band columns; tails use layer-2),
        # then the swap/scan chain (its inputs are long since ready)
        mlp3(qb_t, wq_t, bq_t, NW2, qt_t, l3_cols=max(toff[0], toff[1]))
        emit_swap()

        # band phase per batch: B matmuls -> one PSUM tile; ONE mask-mul;
        # batched po matmuls (lagged 2 batches so they never wait on srun);
        # one staging copy; one DMA per batch.
        ci = 0
        pend = []

        def emit_po(bi, s, k0, ks, wsum):
            nonlocal ci
            o0 = qoff[k0]
            G = len(ks)
            po = ppo.tile([2 * NKC // 2, POMAX], f32, tag="ppo", name="po")
            bspan = bm_t[:, s * NW2 + o0:s * NW2 + o0 + wsum]
            qspan = qt_t[64 * s:64 * s + 64, o0:o0 + wsum]
            nc.tensor.matmul(po[0:2 * G, :wsum],
                             srunb_t[64 * s:64 * s + 64, 2 * k0:2 * k0 + 2 * G],
                             qspan, start=True, stop=False,
                             tile_position=(64 * s, 0))
            nc.tensor.matmul(po[0:2 * G, :wsum],
                             v2st_t[:, 2 * k0:2 * k0 + 2 * G],
                             bspan, start=False, stop=True,
                             tile_position=(0, 0))
            if ci % 3 == 2:
                nc.vector.tensor_copy(stg_t[0:2 * G, bi * POMAX:bi * POMAX + wsum],
                                      po[0:2 * G, :wsum])
            else:
                nc.scalar.copy(stg_t[0:2 * G, bi * POMAX:bi * POMAX + wsum],
                               po[0:2 * G, :wsum])
            ci += 1
            nc.sync.dma_start(d_out[:, bi * POMAX:bi * POMAX + wsum],
                              stg_t[:, bi * POMAX:bi * POMAX + wsum])

        for bi, (s, k0, ks, wsum) in enumerate(plan["batches"]):
            o0 = qoff[k0]
            pBB = pb.tile([C, POMAX], f32, tag="pb", name="pBB")
            for k in ks:
                nq = NB[k]
                o = qoff[k]
                p = kpos[k]
                nc.tensor.matmul(pBB[:, o - o0:o - o0 + nq],
                                 kt_t[64 * s:64 * s + 64, C * p:C * (p + 1)],
                                 qt_t[64 * s:64 * s + 64, o:o + nq],
                                 start=True, stop=True,
                                 tile_position=(64 * s, 0))
            bspan = bm_t[:, s * NW2 + o0:s * NW2 + o0 + wsum]
            nc.vector.tensor_mul(bspan, pBB[:, :wsum],
                                 msk_t[:, s * NW2 + o0:s * NW2 + o0 + wsum])
            pend.append((bi, s, k0, ks, wsum))
            if len(pend) > 2:
                emit_po(*pend.pop(0))
        for args in pend:
            emit_po(*args)

        # tails via layer-2: out2 = a2_block^T @ (Wq3 srun_tot), plus the
        # per-side constant bq3.srun_tot exported in the trailing columns
        pws = paux.tile([2 * D, AUXW], f32, tag="paux", name="pws")
        nc.tensor.matmul(pws[:, 0:2], wq3t_t[:],
                         srunb_t[:, 2 * NKC:2 * NKC + 2], start=True, stop=True)
        wsb_t = work.tile([2 * D, 2], bf16, tag="wsb")
        nc.scalar.copy(wsb_t[:], pws[:, 0:2])
        pt = paux.tile([TB, AUXW], f32, tag="paux", name="pt")
        pt = pt[:, 0:2 * NTBLK + 4]
        for (s, coff, w, tcx) in plan["tblocks"]:
            nc.tensor.matmul(pt[0:w, 2 * tcx:2 * tcx + 2],
                             a2_t[64 * s:64 * s + 64, coff:coff + w],
                             wsb_t[64 * s:64 * s + 64, :],
                             start=True, stop=True, tile_position=(64 * s, 0))
        out2_t = big.tile([TB, 2 * NTBLK + 4], f32, tag="out2")
        nc.scalar.copy(out2_t[:, 0:2 * NTBLK], pt[:, 0:2 * NTBLK])
        nc.scalar.copy(out2_t[:, 2 * NTBLK:2 * NTBLK + 2],
                       srun_t[:, 2 * NKC:2 * NKC + 2])
        nc.vector.memset(out2_t[:, 2 * NTBLK + 2:], 0.0)
        nc.sync.dma_start(d_out2[:], out2_t[:])

    nc.finalize()
    return nc


_CACHE = {}


def kernel(X, wq_w, wq_b, wk_w, wk_b):
    from concourse.bass_utils import run_bass_kernel_spmd

    plan = make_plan(X)
    in_maps = make_inputs(X, wq_w, wq_b, wk_w, wk_b, plan)
    key = (tuple(plan["NB"]), plan["NT"], tuple(map(tuple, plan["tblocks"])))
    if key not in _CACHE:
        _CACHE[key] = build_graph(plan)
    nc = _CACHE[key]
    res = run_bass_kernel_spmd(nc, in_maps, core_ids=list(range(NCORES)),
                               trace=bool(int(os.environ.get("KTRACE", "0"))))
    outs = [res.results[c]["out"] for c in range(NCORES)]
    outs2 = [res.results[c]["out2"] for c in range(NCORES)]
    y = scatter_outputs(plan, outs, outs2, np.asarray(wq_b, np.float32)[NLIN - 1])
    if os.environ.get("KTRACE", "0") != "0":
        kernel.last_result = res
    return y[None]  # (1, T, 2)
